# revision 9
# baseline (speedup 1.0000x reference)
"""BitMultiheadAttention Trainium2 kernel (8 NeuronCores, SPMD).

Sharding (per sharding_hint): tensor-parallel heads x data-parallel batch.
  core c in [0..7]:  b = c // 4 (batch),  g = c % 4 (head-group of 4 heads)

Phase A (per core): subln+int8-quant of x (integer x_int kept exact in bf16),
  ternary weight quant (absmean via tiny AllReduce of shard |w| sums),
  column-parallel QKV for the core's 4 heads (bf16 integer matmuls, exact),
  causal attention for 4 heads (bf16, exp without max-subtraction, Z via a
  ones-column appended to V).
Phase A->B: AllToAll inside each batch group of 4 cores converts the
  head-sharded attention output [2048 tok, 512 dims] into a token-sharded
  full-width slab [512 tok, 2048 dims].
Phase B (per core): subln+quant of its 512 tokens, row matmul against the
  full ternary-quantized wp -> final [512, 2048] f32 output slice.

The host wrapper only shards/gathers numpy arrays; all FLOPs run on device.
"""

import numpy as np
from contextlib import ExitStack

import concourse.bass as bass
import concourse.bass_isa as bass_isa
import concourse.mybir as mybir
import concourse.tile as tile
from concourse import bacc
from concourse.bass_utils import run_bass_kernel_spmd
from concourse.masks import make_identity

F32 = mybir.dt.float32
BF16 = mybir.dt.bfloat16
AX = mybir.AxisListType
ALU = mybir.AluOpType
ACTF = mybir.ActivationFunctionType

S = 2048          # sequence length
D = 2048          # model dim
HD = 128          # head dim
HPG = 4           # heads per core
OG = HPG * HD     # 512 projection dims per core
P = 128           # partitions
NT = S // P       # 16 token tiles
ND = D // P       # 16 contraction tiles
TOKB = S // 4     # 512 tokens per core in phase B
NTB = TOKB // P   # 4
NCORES = 8
MAGIC = float(np.float32(12582912.0))   # 1.5 * 2**23 fp32 round-half-even trick
EPS = 1e-5
ISR = float(np.float32(1.0) / np.float32(np.sqrt(np.float32(HD))))
MASK_NEG = -60.0


def _stats_quant(nc, pools, x_t, ceps, invs_dst, xint_dst, width):
    """subln + per-token int8 absmax quant of one [128, width] f32 tile.

    Writes integer-valued bf16 to xint_dst and absmax/127 to invs_dst [128,1].
    Matches reference rounding: true divides for sigma and 127/absmax, fp32
    magic-number round-half-even.
    """
    v = nc.vector
    sc = nc.scalar
    st = pools["stats"]

    sums = st.tile([P, 1], F32, tag="sums")
    sumsq = st.tile([P, 1], F32, tag="sumsq")
    mx = st.tile([P, 1], F32, tag="mx")
    mn = st.tile([P, 1], F32, tag="mn")
    sqd = pools["sqd"].tile([P, width], F32, tag="sqd")

    v.reduce_sum(sums, x_t, axis=AX.X)
    sc.activation(sqd, x_t, ACTF.Square, accum_out=sumsq)
    v.reduce_max(mx, x_t, axis=AX.X)
    v.tensor_reduce(mn, x_t, axis=AX.X, op=ALU.min)

    mean = st.tile([P, 1], F32, tag="mean")
    var = st.tile([P, 1], F32, tag="var")
    sig = st.tile([P, 1], F32, tag="sig")
    dev = st.tile([P, 1], F32, tag="dev")
    amax = st.tile([P, 1], F32, tag="amax")
    a_sc = st.tile([P, 1], F32, tag="a_sc")
    b_sc = st.tile([P, 1], F32, tag="b_sc")

    v.tensor_scalar_mul(mean, sums, 1.0 / width)
    v.tensor_scalar_mul(var, sumsq, 1.0 / width)          # E[x^2]
    m2 = st.tile([P, 1], F32, tag="m2")
    v.tensor_tensor(m2, mean, mean, ALU.mult)
    v.tensor_tensor(var, var, m2, ALU.subtract)
    sc.activation(sig, var, ACTF.Sqrt, bias=ceps)          # sqrt(var + eps)
    d1 = st.tile([P, 1], F32, tag="d1")
    v.tensor_tensor(d1, mx, mean, ALU.subtract)
    v.tensor_tensor(dev, mean, mn, ALU.subtract)
    v.tensor_tensor(dev, d1, dev, ALU.max)                 # max |x - mean|
    rsig = st.tile([P, 1], F32, tag="rsig")
    v.reciprocal(rsig, sig)                                # IEEE 1/sigma
    v.tensor_tensor(amax, dev, rsig, ALU.mult)             # max |x_norm|
    v.tensor_scalar_max(amax, amax, 1e-5)
    v.tensor_scalar_mul(invs_dst, amax, 1.0 / 127.0)       # dequant LSB
    ramax = st.tile([P, 1], F32, tag="ramax")
    v.reciprocal(ramax, amax)                              # IEEE 1/absmax
    v.tensor_tensor(a_sc, ramax, rsig, ALU.mult)
    v.tensor_scalar_mul(a_sc, a_sc, 127.0)                 # 127/(absmax*sig)
    v.tensor_tensor(b_sc, mean, a_sc, ALU.mult)
    v.tensor_scalar_mul(b_sc, b_sc, -1.0)                  # -mean * a

    y = pools["y"].tile([P, width], F32, tag="y")
    sc.activation(y, x_t, ACTF.Identity, bias=b_sc, scale=a_sc)
    # round-half-even to integer grid, emit bf16 (exact: |int| <= 127)
    v.tensor_scalar(xint_dst, y, MAGIC, MAGIC, ALU.add, ALU.subtract)


def _transpose4(nc, pools, src_bf16, blocks, ident, dst_ap_fn):
    """Transpose `blocks` consecutive [128,128] bf16 blocks of src via PE in
    batches of 4 into one PSUM tile, then one strided copy per batch into the
    destination slab. dst_ap_fn(b0, nb) -> dest AP [P, nb, 128] for blocks
    b0..b0+nb-1."""
    te = nc.tensor
    v = nc.vector
    for b0 in range(0, blocks, 4):
        nb = min(4, blocks - b0)
        pt = pools["ptr"].tile([P, 4 * P], BF16, tag="ptr")
        for j in range(nb):
            te.transpose(pt[:, j * P:(j + 1) * P],
                         src_bf16[:, (b0 + j) * P:(b0 + j + 1) * P], ident)
        v.tensor_copy(out=dst_ap_fn(b0, nb),
                      in_=pt[:, :nb * P].rearrange("p (a b) -> p a b", b=P))


def _build_body(ctx, tc, nc, x_ext, wq_ext, wk_ext, wv_ext, wps_ext, wp_ext,
                out_ext):
    v = nc.vector
    sc = nc.scalar
    gp = nc.gpsimd
    te = nc.tensor
    sy = nc.sync

    dram = ctx.enter_context(tc.tile_pool(name="dram", bufs=1, space="DRAM"))
    a2a_in = dram.tile([S, OG], F32, tag="a2a_in")
    a2a_out = dram.tile([S, OG], F32, tag="a2a_out")
    ar_in = dram.tile([1, 8], F32, tag="ar_in")
    ar_out = dram.tile([1, 8], F32, tag="ar_out")

    const = ctx.enter_context(tc.tile_pool(name="const", bufs=1))
    ident = const.tile([P, P], BF16, tag="ident")
    make_identity(nc, ident)
    ceps = const.tile([P, 1], F32, tag="ceps")
    gp.memset(ceps, EPS)
    zpad = const.tile([1, 4], F32, tag="zpad")
    gp.memset(zpad, 0.0)

    # scale slabs
    scl = ctx.enter_context(tc.tile_pool(name="scl", bufs=1))
    invs = scl.tile([P, NT], F32, tag="invs")
    qs_all = scl.tile([P, NT], F32, tag="qs")
    ks_all = scl.tile([P, NT], F32, tag="ks")
    vs_all = scl.tile([P, NT], F32, tag="vs")
    invsa = scl.tile([P, NTB], F32, tag="invsa")
    pscale = scl.tile([P, NTB], F32, tag="ps")
    amb = scl.tile([P, 8], F32, tag="amb")
    wsum = scl.tile([P, 4], F32, tag="wsum")
    wsum_r = scl.tile([P, 4], F32, tag="wsumr")
    ar_sb = scl.tile([1, 8], F32, tag="arsb")
    amb_src = scl.tile([1, 8], F32, tag="ambsrc")

    pools = {
        "stats": ctx.enter_context(tc.tile_pool(name="stats", bufs=4)),
        "ptr": ctx.enter_context(tc.tile_pool(name="ptr", bufs=3, space="PSUM")),
    }
    ld = ctx.enter_context(tc.tile_pool(name="ld", bufs=2))
    pmm = ctx.enter_context(tc.tile_pool(name="pmm", bufs=3, space="PSUM"))

    # ---------------- stage 0: |w| shard sums -> AllReduce -> am ----------
    gp.memset(wsum, 0.0)
    for wi_x, wext in enumerate([wq_ext, wk_ext, wv_ext, wps_ext]):
        for i in range(OG // P):
            wt = ld.tile([P, D], F32, tag="ld")
            sy.dma_start(wt, wext[i * P:(i + 1) * P, :])
            tmp = pools["stats"].tile([P, 1], F32, tag="wabs")
            v.tensor_reduce(tmp, wt, axis=AX.X, op=ALU.add,
                            apply_absolute_value=True)
            v.tensor_tensor(wsum[:, wi_x:wi_x + 1], wsum[:, wi_x:wi_x + 1],
                            tmp, ALU.add)
    gp.partition_all_reduce(wsum_r, wsum, channels=P,
                            reduce_op=bass_isa.ReduceOp.add)
    sy.dma_start(ar_in[0:1, 0:4], wsum_r[0:1, :])
    sy.dma_start(ar_in[0:1, 4:8], zpad)
    gp.collective_compute(
        "AllReduce",
        ALU.add,
        replica_groups=[list(range(NCORES))],
        ins=[ar_in.opt()],
        outs=[ar_out.opt()],
    )
    sy.dma_start(ar_sb, ar_out[:])
    v.tensor_scalar_mul(amb_src[0:1, 0:4], ar_sb[0:1, 0:4],
                        1.0 / (2.0 * D * D))
    v.tensor_scalar_add(amb_src[0:1, 4:8], amb_src[0:1, 0:4], EPS)
    gp.partition_broadcast(amb, amb_src)
    ramb = scl.tile([P, 4], F32, tag="ramb")
    v.reciprocal(ramb, amb[:, 4:8])          # 1/(am + eps), IEEE

    with ExitStack() as stack_a:
        qt_all = stack_a.enter_context(tc.tile_pool(name="qta", bufs=1)).tile(
            [P, HPG, S], BF16, tag="qta")
        kt_all = stack_a.enter_context(tc.tile_pool(name="kta", bufs=1)).tile(
            [P, HPG, S], BF16, tag="kta")
        v1_all = stack_a.enter_context(tc.tile_pool(name="v1a", bufs=1)).tile(
            [P, NT, HPG, HD + 1], BF16, tag="v1a")
        gp.memset(v1_all[:, :, :, HD:HD + 1], 1.0)

        with ExitStack() as stack_b:
            xT = stack_b.enter_context(tc.tile_pool(name="xt", bufs=1)).tile(
                [P, ND, S], BF16, tag="xT")

            # ---- x pipeline: stats -> quant -> transpose into xT ----
            with ExitStack() as stack_c:
                xpools = dict(pools)
                xpools["sqd"] = stack_c.enter_context(
                    tc.tile_pool(name="sqd", bufs=2))
                xpools["y"] = stack_c.enter_context(
                    tc.tile_pool(name="y", bufs=2))
                xi_pool = stack_c.enter_context(
                    tc.tile_pool(name="xi", bufs=2))
                for tt in range(NT):
                    x_t = ld.tile([P, D], F32, tag="ld")
                    sy.dma_start(x_t, x_ext[tt * P:(tt + 1) * P, :])
                    xint = xi_pool.tile([P, D], BF16, tag="xi")
                    _stats_quant(nc, xpools, x_t, ceps,
                                 invs[:, tt:tt + 1], xint, D)
                    _transpose4(
                        nc, pools, xint, ND, ident,
                        lambda b0, nb, tt=tt: xT[:, b0:b0 + nb,
                                                 tt * P:(tt + 1) * P])

            # dequant scale slabs (need amb + invs)
            v.tensor_tensor(qs_all, invs, amb[:, 0:1].to_broadcast([P, NT]),
                            ALU.mult)
            v.tensor_scalar_mul(qs_all, qs_all, ISR)
            v.tensor_tensor(ks_all, invs, amb[:, 1:2].to_broadcast([P, NT]),
                            ALU.mult)
            v.tensor_tensor(vs_all, invs, amb[:, 2:3].to_broadcast([P, NT]),
                            ALU.mult)

            # ---- weights: quant -> transpose -> QKV matmuls ----
            with ExitStack() as stack_e:
                wt_pool = stack_e.enter_context(
                    tc.tile_pool(name="wt", bufs=1))
                wz_pool = stack_e.enter_context(
                    tc.tile_pool(name="wz", bufs=1))
                wi_pool = stack_e.enter_context(
                    tc.tile_pool(name="wi", bufs=1))
                qk_pool = stack_e.enter_context(
                    tc.tile_pool(name="qk", bufs=2))
                for wi_x, wext in enumerate([wq_ext, wk_ext, wv_ext]):
                    wT = wt_pool.tile([P, ND, OG], BF16, tag="wt")
                    for i in range(OG // P):
                        w_t = ld.tile([P, D], F32, tag="ld")
                        sy.dma_start(w_t, wext[i * P:(i + 1) * P, :])
                        z = wz_pool.tile([P, D], F32, tag="wz")
                        v.tensor_scalar(z, w_t, ramb[:, wi_x:wi_x + 1],
                                        1.49, ALU.mult, ALU.min)
                        v.tensor_scalar(z, z, -1.49, MAGIC, ALU.max, ALU.add)
                        w_int = wi_pool.tile([P, D], BF16, tag="wi")
                        v.tensor_scalar_sub(w_int, z, MAGIC)
                        _transpose4(
                            nc, pools, w_int, ND, ident,
                            lambda b0, nb, i=i: wT[:, b0:b0 + nb,
                                                   i * P:(i + 1) * P])
                    for tt in range(NT):
                        ps = pmm.tile([P, OG], F32, tag="pmm")
                        for dt in range(ND):
                            te.matmul(ps,
                                      lhsT=xT[:, dt, tt * P:(tt + 1) * P],
                                      rhs=wT[:, dt, :], start=(dt == 0),
                                      stop=(dt == ND - 1))
                        if wi_x == 0:
                            qtmp = qk_pool.tile([P, OG], BF16, tag="qk")
                            v.tensor_scalar_mul(qtmp, ps, qs_all[:, tt:tt + 1])
                            _transpose4(
                                nc, pools, qtmp, HPG, ident,
                                lambda b0, nb, tt=tt: qt_all[
                                    :, b0:b0 + nb, tt * P:(tt + 1) * P])
                        elif wi_x == 1:
                            ktmp = qk_pool.tile([P, OG], BF16, tag="qk")
                            v.tensor_scalar_mul(ktmp, ps, ks_all[:, tt:tt + 1])
                            _transpose4(
                                nc, pools, ktmp, HPG, ident,
                                lambda b0, nb, tt=tt: kt_all[
                                    :, b0:b0 + nb, tt * P:(tt + 1) * P])
                        else:
                            v.tensor_scalar_mul(
                                v1_all[:, tt, :, :HD],
                                ps.rearrange("p (a b) -> p a b", b=HD),
                                vs_all[:, tt:tt + 1])

        # ---- wp quant + transpose (emitted early so its DVE/DMA work
        # overlaps attention's PE work) ----
        with ExitStack() as stack_w:
            wpT = stack_w.enter_context(tc.tile_pool(name="wpt", bufs=1)).tile(
                [P, ND, D], BF16, tag="wpT")
            wz2_pool = stack_w.enter_context(tc.tile_pool(name="wz2", bufs=1))
            wi2_pool = stack_w.enter_context(tc.tile_pool(name="wi2", bufs=1))
            for i in range(ND):
                w_t = ld.tile([P, D], F32, tag="ld")
                sy.dma_start(w_t, wp_ext[i * P:(i + 1) * P, :])
                z = wz2_pool.tile([P, D], F32, tag="wz2")
                v.tensor_scalar(z, w_t, ramb[:, 3:4], 1.49, ALU.mult,
                                ALU.min)
                v.tensor_scalar(z, z, -1.49, MAGIC, ALU.max, ALU.add)
                w_int = wi2_pool.tile([P, D], BF16, tag="wi2")
                v.tensor_scalar_sub(w_int, z, MAGIC)
                _transpose4(
                    nc, pools, w_int, ND, ident,
                    lambda b0, nb, i=i: wpT[:, b0:b0 + nb,
                                            i * P:(i + 1) * P])

            # ---- attention ----
            with ExitStack() as stack_f1:
                mask_pool = stack_f1.enter_context(
                    tc.tile_pool(name="mask", bufs=1))
                masks = []
                for j in range(4):
                    m = mask_pool.tile([P, 4 * P], BF16, tag=f"mask{j}")
                    gp.memset(m, 0.0)
                    # keep 0 where q - k - 128*j >= 0, else add MASK_NEG
                    gp.affine_select(out=m, in_=m, compare_op=ALU.is_ge,
                                     fill=MASK_NEG, base=-P * j,
                                     pattern=[[1, 4 * P]],
                                     channel_multiplier=-1)
                    masks.append(m)

                pt_pool = stack_f1.enter_context(
                    tc.tile_pool(name="pt", bufs=18))
                pav_pool = stack_f1.enter_context(
                    tc.tile_pool(name="pav", bufs=2, space="PSUM"))
                st_pool = stack_f1.enter_context(
                    tc.tile_pool(name="sta", bufs=3))

                for h in range(HPG):
                    for qc in range(4):
                        ktmax = 4 * (qc + 1)
                        pts = []
                        for kt in range(ktmax):
                            ps = pmm.tile([P, 4 * P], F32, tag="pmm")
                            te.matmul(
                                ps, lhsT=kt_all[:, h, kt * P:(kt + 1) * P],
                                rhs=qt_all[:, h, qc * 4 * P:(qc + 1) * 4 * P],
                                start=True, stop=True)
                            j = kt - 4 * qc
                            if j >= 0:
                                v.tensor_tensor(ps, ps, masks[j], ALU.add)
                            pt_t = pt_pool.tile([P, 4 * P], BF16, tag="pt")
                            sc.activation(pt_t, ps, ACTF.Exp)
                            pts.append(pt_t)
                        for qi in range(4):
                            qt_i = qc * 4 + qi
                            pav = pav_pool.tile([P, HD + 1], F32, tag="pav")
                            for kt in range(ktmax):
                                te.matmul(
                                    pav,
                                    lhsT=pts[kt][:, qi * P:(qi + 1) * P],
                                    rhs=v1_all[:, kt, h, :],
                                    start=(kt == 0), stop=(kt == ktmax - 1))
                            zr = pools["stats"].tile([P, 1], F32, tag="zr")
                            v.reciprocal(zr, pav[:, HD:HD + 1])
                            st_t = st_pool.tile([P, HD], F32, tag="sta")
                            v.tensor_scalar_mul(st_t, pav[:, :HD], zr)
                            sy.dma_start(
                                a2a_in[qt_i * P:(qt_i + 1) * P,
                                       h * HD:(h + 1) * HD], st_t)

            # ---------------- A2A ----------------------------------------
            # 8-core AllToAll: block j of a2a_in = attn tokens
            # [256j, 256j+256) of this core's batch, this core's 512 dims.
            # Core j receives, from all 8 srcs, BOTH batches x all 4
            # dim-groups of token range [256j, 256j+256): a full
            # [512 tok-rows, 2048] phase-B slab.
            gp.collective_compute(
                "AllToAll",
                ALU.bypass,
                replica_groups=[list(range(NCORES))],
                ins=[a2a_in.opt()],
                outs=[a2a_out.opt()],
            )

            # ---------------- phase B -------------------------------------
            with ExitStack() as stack_f2:
                aT = stack_f2.enter_context(
                    tc.tile_pool(name="ata", bufs=1)).tile(
                        [P, ND, TOKB], BF16, tag="aT")
                apools = dict(pools)
                apools["sqd"] = stack_f2.enter_context(
                    tc.tile_pool(name="sqd2", bufs=1))
                apools["y"] = stack_f2.enter_context(
                    tc.tile_pool(name="y2", bufs=1))
                ai_pool = stack_f2.enter_context(
                    tc.tile_pool(name="ai", bufs=1))
                os_pool = stack_f2.enter_context(
                    tc.tile_pool(name="os", bufs=2))

                # a2a_out rows: [src c' = 4*b + g'][256 tokens][512 dims].
                # Tile tt: batch b = tt // 2, offset = (tt % 2) * 128.
                for tt in range(NTB):
                    a_t = ld.tile([P, D], F32, tag="ld")
                    b2 = tt // 2
                    off = (tt % 2) * P
                    for g2 in range(4):
                        base = (b2 * 4 + g2) * (S // NCORES) + off
                        sy.dma_start(a_t[:, g2 * OG:(g2 + 1) * OG],
                                     a2a_out[base:base + P, :])
                    aint = ai_pool.tile([P, D], BF16, tag="ai")
                    _stats_quant(nc, apools, a_t, ceps,
                                 invsa[:, tt:tt + 1], aint, D)
                    _transpose4(
                        nc, pools, aint, ND, ident,
                        lambda b0, nb, tt=tt: aT[:, b0:b0 + nb,
                                                 tt * P:(tt + 1) * P])

                v.tensor_tensor(pscale, invsa,
                                amb[:, 3:4].to_broadcast([P, NTB]), ALU.mult)

                for tt in range(NTB):
                    for oc in range(4):
                        ps = pmm.tile([P, 4 * P], F32, tag="pmm")
                        for dt in range(ND):
                            te.matmul(
                                ps, lhsT=aT[:, dt, tt * P:(tt + 1) * P],
                                rhs=wpT[:, dt, oc * 4 * P:(oc + 1) * 4 * P],
                                start=(dt == 0), stop=(dt == ND - 1))
                        o_t = os_pool.tile([P, 4 * P], F32, tag="os")
                        v.tensor_scalar_mul(o_t, ps, pscale[:, tt:tt + 1])
                        sy.dma_start(
                            out_ext[tt * P:(tt + 1) * P,
                                    oc * 4 * P:(oc + 1) * 4 * P], o_t)


_CACHE = {}


def _get_program():
    if "nc" not in _CACHE:
        nc = bacc.Bacc("TRN2", target_bir_lowering=False, debug=False,
                       num_devices=NCORES)
        x_ext = nc.dram_tensor("x", [S, D], F32, kind="ExternalInput").ap()
        wq_ext = nc.dram_tensor("wq", [OG, D], F32, kind="ExternalInput").ap()
        wk_ext = nc.dram_tensor("wk", [OG, D], F32, kind="ExternalInput").ap()
        wv_ext = nc.dram_tensor("wv", [OG, D], F32, kind="ExternalInput").ap()
        wps_ext = nc.dram_tensor("wps", [OG, D], F32,
                                 kind="ExternalInput").ap()
        wp_ext = nc.dram_tensor("wp", [D, D], F32, kind="ExternalInput").ap()
        out_ext = nc.dram_tensor("out", [TOKB, D], F32,
                                 kind="ExternalOutput").ap()
        with tile.TileContext(nc) as tc:
            with ExitStack() as ctx:
                _build_body(ctx, tc, nc, x_ext, wq_ext, wk_ext, wv_ext,
                            wps_ext, wp_ext, out_ext)
        nc.compile()
        _CACHE["nc"] = nc
    return _CACHE["nc"]


def _in_maps(inputs):
    x = np.ascontiguousarray(np.asarray(inputs["x"], dtype=np.float32))
    ws = {k: np.ascontiguousarray(np.asarray(inputs[k], dtype=np.float32))
          for k in ("wq", "wk", "wv", "wp")}
    maps = []
    for c in range(NCORES):
        b, g = c // 4, c % 4
        maps.append({
            "x": x[b],
            "wq": np.ascontiguousarray(ws["wq"][g * OG:(g + 1) * OG]),
            "wk": np.ascontiguousarray(ws["wk"][g * OG:(g + 1) * OG]),
            "wv": np.ascontiguousarray(ws["wv"][g * OG:(g + 1) * OG]),
            "wps": np.ascontiguousarray(ws["wp"][g * OG:(g + 1) * OG]),
            "wp": ws["wp"],
        })
    return maps


def _run(inputs, trace=False, **kwargs):
    nc = _get_program()
    res = run_bass_kernel_spmd(nc, _in_maps(inputs),
                               core_ids=list(range(NCORES)), trace=trace,
                               **kwargs)
    B = 2
    TOK8 = S // NCORES  # 256 tokens per (core, batch)
    out = np.zeros((B, S, D), dtype=np.float32)
    for c in range(NCORES):
        oc = res.results[c]["out"]
        out[0, c * TOK8:(c + 1) * TOK8, :] = oc[:TOK8]
        out[1, c * TOK8:(c + 1) * TOK8, :] = oc[TOK8:]
    return out, res


def kernel(**inputs):
    out, _ = _run(inputs)
    return out


if __name__ == "__main__":
    import sys
    if "--build" in sys.argv:
        _get_program()
        print("build + compile OK")
